# revision 64
# baseline (speedup 1.0000x reference)
"""Trainium2 Bass kernel for local-window multi-head self-attention (v6).

Problem shape (hardcoded): B=16, H=8, W=128 -> N=1024, C=768, nh=8, hd=96,
local window 7x11 (|dh|<=3, |dw|<=5).

v6 = v3 (w-major band attention, host-side transposes, hand interleave)
plus full-width PE tiling and a reworked pipeline:
  - Q/K projection runs with M=128 stationary tiles (12 row-groups over
    the 1536 q|k rows, single full-width eviction each) instead of
    per-head M=96 tiles: 73.7k instead of 98.3k PE columns per batch.
  - heads whose 96 q/k rows misalign with the 128-row packing are
    DMA-repacked (SBUF->SBUF, partition shift) into base-0 [96, N] tiles
    one head ahead of use, so every score matmul stays one instruction.
  - attention output is normalized into six packed [128, N] tiles with
    quadrant-legal DVE pieces; the output projection contracts over 6
    full 128-row chunks (73.7k -> 61.4k... 6 vs 8 matmuls per chunk).
  - head->outC slot permutation (wpT rows permuted on host to match)
    gives the late-emitted heads the 1-piece normalization slots, so the
    kernel tail never waits long DVE chains.
  - normalization multiplies are deferred one head (the reciprocal ->
    Pool broadcast chain never head-of-line blocks the DVE queue).
  - schedule: per-head ATT interleaves one qk unit (produced one position
    ahead of need) + spread PROJ(prev batch) chunks; all 16 V groups of
    the next batch fill positions 4-7; the final PROJ is software
    pipelined across 4 PSUM tiles and 4 output staging buffers.

Sharding: data-parallel over B across 8 NeuronCores (2 batches per core).
"""

import sys

sys.path.insert(0, "/opt/trn_rl_repo")

import numpy as np

import concourse.bacc as bacc
import concourse.mybir as mybir
import concourse.tile as tile
from concourse.bass_utils import run_bass_kernel_spmd

F32R = mybir.dt.float32r
F32 = mybir.dt.float32
BF16 = mybir.dt.bfloat16
AF = mybir.ActivationFunctionType

B, H, W, C = 16, 8, 128, 768
N = H * W  # 1024
NH, HD = 8, 96
NCORES = 8
BLOC = B // NCORES  # batches per core
SCALE = float(HD) ** -0.5
DH, DW = 3, 5  # |dh|<=3 rows, |dw|<=5 cols
QLO, QHI = 8 * DW, 128 + 8 * DW  # query window [128k-40, 128k+168)
MW = QLO + QHI  # mask width 208


def _att_blocks():
    """Emission-ordered key blocks: [(k, q0, q1, pieces)], order [0,1,7,2..6].

    k=0 (k=7) opens bank A (B) with a full-width 512-col AV matmul (its exm
    tile is zero-padded) so the start=True matmul covers the whole bank.
    Each piece: (c0, c1, half, start, stop) -- absolute query cols [c0, c1).
    """
    order = [0, 1, 7, 2, 3, 4, 5, 6]
    raw = {}
    for k in range(8):
        q0 = max(0, 128 * k - QLO)
        q1 = min(N, 128 * k + QHI)
        pieces = []
        if q0 < 512:
            pieces.append([q0, min(q1, 512), 0])
        if q1 > 512:
            pieces.append([max(q0, 512), q1, 1])
        raw[k] = (q0, q1, pieces)
    last_pos = {}
    for pos, k in enumerate(order):
        for _c0, _c1, half in raw[k][2]:
            last_pos[half] = pos
    blocks = []
    first = {0: True, 1: True}
    for pos, k in enumerate(order):
        q0, q1, pieces = raw[k]
        out = []
        for c0, c1, half in pieces:
            if first[half]:
                out.append((512 * half, 512 * half + 512, half, True, pos == last_pos[half]))
                first[half] = False
            else:
                out.append((c0, c1, half, False, pos == last_pos[half]))
        blocks.append((k, q0, q1, out))
    return blocks


ATT_BLOCKS = _att_blocks()


def build_nc():
    nc = bacc.Bacc(None, target_bir_lowering=False)
    xT_d = nc.dram_tensor("xT", [BLOC, C, N], BF16, kind="ExternalInput")
    wT_d = nc.dram_tensor("wT", [C, 3 * C], BF16, kind="ExternalInput")
    wpT_d = nc.dram_tensor("wpT", [C, C], BF16, kind="ExternalInput")
    bias_d = nc.dram_tensor("bias", [C], F32, kind="ExternalInput")
    mask_d = nc.dram_tensor("maskband", [128, MW], BF16, kind="ExternalInput")
    yT_d = nc.dram_tensor("yT", [BLOC, C, N], F32, kind="ExternalOutput")
    _emit_body(nc, xT_d, wT_d, wpT_d, bias_d, mask_d, yT_d)
    nc.finalize()
    return nc


def _emit_body(nc, xT_d, wT_d, wpT_d, bias_d, mask_d, yT_d):
    with tile.TileContext(nc) as tc:
        with (
            tc.tile_pool(name="const", bufs=1) as constp,
            tc.tile_pool(name="wperm", bufs=1) as wpermp,
            tc.tile_pool(name="xpool", bufs=2) as xp,
            tc.tile_pool(name="qkpool", bufs=1) as qkp,
            tc.tile_pool(name="vpool", bufs=2) as vp,
            tc.tile_pool(name="outp", bufs=2) as outp,
            tc.tile_pool(name="work", bufs=2) as workp,
            tc.tile_pool(name="ypool", bufs=4) as yp,
            tc.tile_pool(name="mmps", bufs=2, space="PSUM") as mmps,
            tc.tile_pool(name="scps", bufs=2, space="PSUM") as scps,
            tc.tile_pool(name="avps", bufs=2, space="PSUM") as avps,
        ):
            # ---- constants ----
            mask = constp.tile([128, MW], BF16, tag="mask", name="mask")
            nc.sync.dma_start(mask[:], mask_d[:])
            bias = constp.tile([128, 6], F32, tag="bias", name="bias")
            nc.sync.dma_start(bias[:], bias_d.ap().rearrange("(j p) -> p j", p=128))

            # zero-padded exm tiles for the bank-opening AV matmuls (k=0, k=7)
            exm_pad = [
                workp.tile([128, 512], BF16, tag=f"exmpad{i}", name=f"exmpad{i}", bufs=1)
                for i in range(2)
            ]
            nc.gpsimd.memset(exm_pad[0][:, QHI - QLO :], 0.0)
            nc.gpsimd.memset(exm_pad[1][:, : 512 - (QHI - QLO)], 0.0)

            # ---- weights: direct DMA of host-transposed layouts ----
            wT = [wpermp.tile([128, 3 * C], BF16, tag=f"wT{c}", name=f"wT{c}") for c in range(6)]
            wpT = [wpermp.tile([128, C], BF16, tag=f"wpT{j}", name=f"wpT{j}") for j in range(6)]

            def stage_x(b, with_wv=False):
                """Stage xT in token halves (and wv in ng halves) so the first
                DMA wave [x half0 + wv ng0] unblocks 8 full V groups at once
                instead of the c-loop crawling behind a serial DMA queue."""
                xT = [xp.tile([128, N], BF16, tag=f"xT{c}", name=f"xT{c}") for c in range(6)]
                for c in range(6):
                    nc.sync.dma_start(
                        xT[c][:, 0:512], xT_d[b, 128 * c : 128 * (c + 1), 0:512]
                    )
                    if with_wv:
                        nc.sync.dma_start(
                            wT[c][:, 2 * C : 2 * C + 384],
                            wT_d[128 * c : 128 * (c + 1), 2 * C : 2 * C + 384],
                        )
                for c in range(6):
                    nc.sync.dma_start(
                        xT[c][:, 512:1024], xT_d[b, 128 * c : 128 * (c + 1), 512:1024]
                    )
                if with_wv:
                    for c in range(6):
                        nc.sync.dma_start(
                            wT[c][:, 2 * C + 384 :],
                            wT_d[128 * c : 128 * (c + 1), 2 * C + 384 :],
                        )
                return xT

            def load_weights():
                for c in range(6):
                    nc.sync.dma_start(
                        wT[c][:, : 2 * C], wT_d[128 * c : 128 * (c + 1), : 2 * C]
                    )
                for j in range(6):
                    nc.sync.dma_start(wpT[j][:], wpT_d[128 * j : 128 * (j + 1), :])

            def v_groups(xT):
                """16 closures, each a 6-MM group computing one v_sb chunk."""
                v_sb = vp.tile([128, 8 * NH * 97], BF16, tag="v", name="v")
                ones_ap = v_sb[:].rearrange("p (t e) -> p t e", t=64)[:, :, 96:97]
                nc.gpsimd.memset(ones_ap, 1.0)
                groups = []
                for ng in range(2):
                    for t in range(8):
                        def g(t=t, ng=ng):
                            pv = mmps.tile([128, 384], F32, tag="mm", name="mm")
                            for c in range(6):
                                nc.tensor.matmul(
                                    pv[:],
                                    xT[c][:, 128 * t : 128 * (t + 1)],
                                    wT[c][:, 2 * C + 384 * ng : 2 * C + 384 * (ng + 1)],
                                    start=(c == 0),
                                    stop=(c == 5),
                                )
                            out_ap = v_sb[:].rearrange("p (t h e) -> p t h e", t=8, h=NH)[
                                :, t, 4 * ng : 4 * (ng + 1), 0:96
                            ]
                            nc.scalar.copy(
                                out_ap, pv[:].rearrange("p (h e) -> p h e", h=4)
                            )
                        groups.append(g)
                return v_sb, groups

            def qk_tiles():
                """12 packed [128, N] bf16 tiles holding the q|k rows
                [128g, 128g+128); aligned heads (base 0) are read directly,
                misaligned heads through the DMA-repacked qa/ka tiles."""
                return [
                    qkp.tile([128, N], BF16, tag=f"qk{g}", name=f"qk{g}")
                    for g in range(12)
                ]

            STRADDLE = (1, 2, 3, 5, 6, 7)  # heads whose rows straddle/misalign

            # head -> outC slot, by normalization piece count (slot base
            # 0/32/64/96 -> 1/3/2/3 quadrant pieces): late-emitted heads get
            # the cheap slots (h7,h4 -> 1 piece; h5,h6 -> 2) so the tail DVE
            # queue stays short; wpT rows are permuted to match on the host.
            SLOT = {7: 0, 0: 1, 5: 2, 1: 3, 4: 4, 2: 5, 6: 6, 3: 7}

            def qa_tiles():
                qa = {h: qkp.tile([HD, N], BF16, tag=f"qa{h}", name=f"qa{h}") for h in STRADDLE}
                ka = {h: qkp.tile([HD, N], BF16, tag=f"ka{h}", name=f"ka{h}") for h in STRADDLE}
                return qa, ka

            def emit_repack(h, qk_sb, qa, ka):
                """DMA the misaligned q/k rows of head h out of the packed
                tiles into partition-base-0 [96, N] tiles (Act HWDGE queue,
                issued one full head ahead of use)."""
                for dst, base_e in ((qa[h], 96 * h), (ka[h], 768 + 96 * h)):
                    r = base_e
                    while r < base_e + 96:
                        g = r // 128
                        r1 = min(base_e + 96, 128 * (g + 1))
                        nc.sync.dma_start(
                            dst[r - base_e : r1 - base_e, :],
                            qk_sb[g][r - 128 * g : r1 - 128 * g, :],
                        )
                        r = r1

            def qk_group_closures(xT, qk_sb, g):
                """Two closures (one per token half): 6-MM M=128 row-group g of
                the packed q|k rows, evicted with one full-width copy."""
                out = []
                for half in (0, 1):
                    def go(half=half, g=g):
                        pq = mmps.tile([128, 512], F32, tag="mm", name="mm")
                        for c in range(6):
                            nc.tensor.matmul(
                                pq[:],
                                wT[c][:, 128 * g : 128 * (g + 1)],
                                xT[c][:, 512 * half : 512 * (half + 1)],
                                start=(c == 0),
                                stop=(c == 5),
                            )
                        nc.scalar.copy(
                            qk_sb[g][:, 512 * half : 512 * (half + 1)], pq[:]
                        )
                    out.append(go)
                return out

            def proj_closures(b, outC):
                """12 closures: output projection chunk (e, half) with K=128
                contraction over the 6 packed outC tiles. half-major order so
                the half-0 chunks never wait on NORM(*, 1); bias-add eviction
                alternates Act/DVE to halve eviction back-pressure."""
                out = []
                for half in range(2):
                    for e in range(6):
                        def go(e=e, half=half, outC=outC, b=b):
                            py = mmps.tile([128, 512], F32, tag="mm", name="mm")
                            # j=0/3 last: finished by the tail heads (h7/h4)
                            for ji, j in enumerate((1, 2, 4, 5, 0, 3)):
                                nc.tensor.matmul(
                                    py[:],
                                    wpT[j][:, 128 * e : 128 * (e + 1)],
                                    outC[j][:, 512 * half : 512 * (half + 1)],
                                    start=(ji == 0),
                                    stop=(ji == 5),
                                )
                            yt = yp.tile([128, 512], F32, tag="yt", name="yt")
                            nc.scalar.add(yt[:], py[:], bias[:, e : e + 1])
                            nc.sync.dma_start(
                                yT_d[b, 128 * e : 128 * (e + 1), 512 * half : 512 * (half + 1)],
                                yt[:],
                            )
                        out.append(go)
                return out

            def emit_att(h, qsrc, ksrc, v_sb, outC, fillers, pending, last=False):
                """ATT(b,h) with filler closures spliced into the PE stream.

                The outC normalization multiplies of head h-1 arrive as
                `pending` and are emitted late in this head's sequence, after
                this head's mask-muls: the reciprocal -> Pool broadcast chain
                then never head-of-line blocks the DVE queue. Returns this
                head's deferred multiplies."""
                av = [avps.tile([97, 512], F32, tag=f"av{i}", name=f"av{i}") for i in range(2)]
                sc_t = {}
                exm_t = {}
                qt, qr = qsrc
                kt, kr = ksrc

                def S(j):
                    k, q0, q1, _p = ATT_BLOCKS[j]
                    wq = q1 - q0
                    mo = q0 - (128 * k - QLO)
                    sc = scps.tile([128, 256], F32, tag="sc", name="sc")
                    sc_t[j] = sc
                    nc.tensor.matmul(
                        sc[:, :wq],
                        kt[kr : kr + HD, 128 * k : 128 * (k + 1)],
                        qt[qr : qr + HD, q0:q1],
                        start=True,
                        stop=True,
                    )
                    ex = workp.tile([128, 256], BF16, tag="ex", name="ex", bufs=3)
                    nc.scalar.activation(ex[:, :wq], sc[:, :wq], AF.Exp, scale=SCALE)
                    if k in (0, 7):
                        exm = exm_pad[0 if k == 0 else 1]
                        eo = q0 - 512 * (k == 7)
                    else:
                        exm = workp.tile([128, 256], BF16, tag="exm", name="exm", bufs=3)
                        eo = 0
                    nc.vector.tensor_mul(
                        exm[:, eo : eo + wq], ex[:, :wq], mask[:, mo : mo + wq]
                    )
                    exm_t[j] = (exm, eo)

                def A(j):
                    k, q0, q1, pieces = ATT_BLOCKS[j]
                    exm, eo = exm_t[j]
                    vs = v_sb[:].rearrange("p (t e) -> p t e", t=64)[:, k * NH + h, :]
                    for c0, c1, half, start, stop in pieces:
                        if k in (0, 7):
                            rhs = exm[:, c0 - 512 * half : c1 - 512 * half]
                        else:
                            rhs = exm[:, c0 - q0 + eo : c1 - q0 + eo]
                        nc.tensor.matmul(
                            av[half][:, c0 - 512 * half : c1 - 512 * half],
                            vs,
                            rhs,
                            start=start,
                            stop=stop,
                        )

                deferred = []

                def NORM_pre(half):
                    rec = workp.tile([1, 512], F32, tag="rec", name="rec")
                    nc.vector.reciprocal(rec[:], av[half][96:97, :])
                    recb = workp.tile([HD, 512], F32, tag="recb", name="recb", bufs=3)
                    nc.gpsimd.partition_broadcast(recb[:], rec[:])

                    CAP = {0: 128, 32: 32, 64: 64, 96: 32}
                    s = SLOT[h]

                    def muls(half=half, recb=recb, av=av, outC=outC, s=s):
                        r = 96 * s
                        while r < 96 * (s + 1):
                            j = r // 128
                            ob, ib = r - 128 * j, r - 96 * s
                            step = min(
                                96 * (s + 1) - r, 128 * (j + 1) - r, CAP[ob], CAP[ib]
                            )
                            nc.vector.tensor_mul(
                                outC[j][ob : ob + step, 512 * half : 512 * (half + 1)],
                                av[half][ib : ib + step, :],
                                recb[ib : ib + step, :],
                            )
                            r += step

                    deferred.append(muls)

                fi = iter(fillers)

                def F():
                    g = next(fi, None)
                    if g is not None:
                        g()

                def PENDING():
                    for m in pending:
                        m()

                def PEND1(i):
                    if i < len(pending):
                        pending[i]()

                seq = [
                    lambda: S(0), lambda: S(1), F, lambda: A(0),
                    lambda: S(2), F, lambda: A(1),
                    lambda: S(3), F, lambda: A(2),
                    lambda: S(4), lambda: A(3),
                    lambda: S(5), F, lambda: A(4),
                    lambda: S(6), lambda: S(7), F, lambda: A(5),
                ]
                if not last:
                    seq += [
                        PENDING, lambda: NORM_pre(0), lambda: A(6), lambda: A(7),
                        lambda: NORM_pre(1),
                    ]
                else:
                    # interleave: the reciprocal/broadcast chains overlap the
                    # pending muls; each half's outC writes land ASAP
                    seq += [
                        lambda: NORM_pre(0), lambda: PEND1(0), lambda: deferred[0](),
                        lambda: A(6), lambda: A(7),
                        lambda: NORM_pre(1), lambda: PEND1(1), lambda: deferred[1](),
                    ]
                for step in seq:
                    step()
                # drain any unused fillers
                for g in fi:
                    g()
                return [] if last else deferred

            # ================= main schedule =================
            xT = stage_x(0, with_wv=True)
            load_weights()
            v_sb, vgs = v_groups(xT)
            for g in vgs:
                g()
            next_xT = None
            next_v = None
            prev_proj = None
            pending = []
            for b in range(BLOC):
                if b > 0:
                    xT, v_sb = next_xT, next_v
                qk_sb = qk_tiles()
                qa, ka = qa_tiles()
                outC = [
                    outp.tile([128, N], BF16, tag=f"outC{j}", name=f"outC{j}")
                    for j in range(6)
                ]
                for g0 in (0, 6, 1, 7):
                    for go in qk_group_closures(xT, qk_sb, g0):
                        go()
                if b + 1 < BLOC:
                    next_xT = stage_x(b + 1)
                # Head emission order: h4 last (its 96 rows sit at partition
                # base 0 of outC3, so the tail normalization is one piece).
                # Position p's unit produces the qk tiles position p+2 needs;
                # position p also issues the repack DMAs for position p+1.
                HEAD_ORDER = (0, 1, 2, 3, 5, 6, 7, 4)
                UNITS = {0: (2, 8), 1: (3, 9), 2: (4, 10), 3: (5, 11)}
                # PROJ(b-1) chunks spread over positions 1-7 (position 0 emits
                # the deferred h7 normalization PENDING, which PROJ(b-1) reads)
                pchunks = {0: [], 1: [0, 1], 2: [2], 3: [3], 4: [4, 5],
                           5: [6, 7], 6: [8, 9], 7: [10, 11]}

                def src(h):
                    if h in qa:
                        return (qa[h], 0), (ka[h], 0)
                    g = (96 * h) // 128
                    return (qk_sb[g], (96 * h) % 128), (qk_sb[g + 6], (96 * h) % 128)

                for pos, h in enumerate(HEAD_ORDER):
                    if pos + 1 < NH and HEAD_ORDER[pos + 1] in qa:
                        emit_repack(HEAD_ORDER[pos + 1], qk_sb, qa, ka)
                    if pos < 4:
                        g1, g2 = UNITS[pos]
                        fillers = qk_group_closures(xT, qk_sb, g1) + qk_group_closures(
                            xT, qk_sb, g2
                        )
                        if prev_proj is not None:
                            fillers += [prev_proj[i] for i in pchunks[pos]]
                    elif b + 1 < BLOC:
                        if pos == 4:
                            next_v, nvgs = v_groups(next_xT)
                            fillers = nvgs[:4]
                        elif pos == 5:
                            fillers = nvgs[4:9]
                        elif pos == 6:
                            fillers = nvgs[9:13]
                        else:
                            fillers = nvgs[13:]
                    elif prev_proj is not None:
                        fillers = [prev_proj[i] for i in pchunks[pos]]
                    else:
                        fillers = []
                    last = b + 1 == BLOC and pos == 7
                    qsrc, ksrc = src(h)
                    pending = emit_att(
                        h, qsrc, ksrc, v_sb, outC, fillers, pending, last=last
                    )
                if b + 1 < BLOC:
                    prev_proj = proj_closures(b, outC)
                else:
                    # software-pipelined final PROJ: each chunk's j=3 matmul
                    # (gated on the last head's normalization) trails one
                    # chunk of independent work
                    chunks = [(half, e) for half in range(2) for e in range(6)]
                    ptile = {}

                    def PA(i):
                        half, e = chunks[i]
                        # alternate between the mm pool and the (now idle) av
                        # pool: 4 PSUM tiles deep, eviction latency fully hidden
                        if i % 2 == 0:
                            py = mmps.tile([128, 512], F32, tag="mm", name="mm")
                        else:
                            py = avps.tile([128, 512], F32, tag="av0", name="av0")
                        ptile[i] = py
                        for ji, j in enumerate((1, 2, 4, 5)):
                            nc.tensor.matmul(
                                py[:],
                                wpT[j][:, 128 * e : 128 * (e + 1)],
                                outC[j][:, 512 * half : 512 * (half + 1)],
                                start=(ji == 0),
                                stop=False,
                            )

                    def PB(i):
                        half, e = chunks[i]
                        py = ptile.pop(i)
                        for ji, j in enumerate((0, 3)):
                            nc.tensor.matmul(
                                py[:],
                                wpT[j][:, 128 * e : 128 * (e + 1)],
                                outC[j][:, 512 * half : 512 * (half + 1)],
                                start=False,
                                stop=(ji == 1),
                            )
                        yt = yp.tile([128, 512], F32, tag="yt", name="yt")
                        if i % 2 == 0:
                            nc.scalar.add(yt[:], py[:], bias[:, e : e + 1])
                        else:
                            nc.vector.tensor_scalar_add(yt[:], py[:], bias[:, e : e + 1])
                        nc.sync.dma_start(
                            yT_d[b, 128 * e : 128 * (e + 1), 512 * half : 512 * (half + 1)],
                            yt[:],
                        )

                    PA(0)
                    for i in range(1, 12):
                        PA(i)
                        PB(i - 1)
                    PB(11)


_NC_CACHE = {}


def _get_nc():
    if "nc" not in _NC_CACHE:
        _NC_CACHE["nc"] = build_nc()
    return _NC_CACHE["nc"]


def _bass_kernel(nc, xT, wT, wpT, bias, maskband):
    yT_d = nc.dram_tensor("yT", [BLOC, C, N], F32, kind="ExternalOutput")
    _emit_body(nc, xT, wT, wpT, bias, maskband, yT_d)
    return yT_d


def _get_runner():
    if "fn" in _NC_CACHE:
        return _NC_CACHE["fn"], _NC_CACHE["mesh"]
    import jax
    from jax.experimental.shard_map import shard_map
    from jax.sharding import Mesh, PartitionSpec

    from concourse.bass2jax import bass_jit

    kern = bass_jit(_bass_kernel)
    devices = jax.devices()[:NCORES]
    mesh = Mesh(np.asarray(devices), ("core",))
    P = PartitionSpec
    fn = jax.jit(
        shard_map(
            kern,
            mesh=mesh,
            in_specs=(P("core"),) * 5,
            out_specs=P("core"),
            check_rep=False,
        )
    )
    _NC_CACHE["fn"] = fn
    _NC_CACHE["mesh"] = mesh
    return fn, mesh


# outC slot s holds head _SLOT_HEADS[s]'s output dims (see SLOT in the
# kernel body); wpT rows are permuted to match.
_SLOT_HEADS = (7, 0, 5, 1, 4, 2, 6, 3)
_WPT_PERM = np.concatenate([np.arange(96 * h, 96 * (h + 1)) for h in _SLOT_HEADS])


def _band_mask():
    """[128, 208] bf16: mask[i, j] for key i in block, query offset r=j-40."""
    import ml_dtypes

    i = np.arange(128)
    r = np.arange(-QLO, QHI)
    wk, hk = i // 8, i % 8
    wq, hq = np.floor_divide(r, 8), np.mod(r, 8)
    m = (np.abs(wk[:, None] - wq[None, :]) <= DW) & (
        np.abs(hk[:, None] - hq[None, :]) <= DH
    )
    return m.astype(np.float32).astype(ml_dtypes.bfloat16)


def _prep_xT(x):
    """[Bn, N, C] row-major tokens -> [Bn, C, N'] with w-major tokens."""
    Bn = x.shape[0]
    return np.ascontiguousarray(
        x.reshape(Bn, H, W, C).transpose(0, 3, 2, 1).reshape(Bn, C, N)
    )


def _unpermute_y(y):
    """w-major tokens back to row-major."""
    Bn = y.shape[0]
    return np.ascontiguousarray(
        y.reshape(Bn, W, H, C).transpose(0, 2, 1, 3).reshape(Bn, N, C)
    )


def global_inputs(x, w_qkv, w_proj, b_proj):
    """Pre-process + concatenate per-core inputs along axis 0 for shard_map."""
    import ml_dtypes

    xT_g = _prep_xT(x).reshape(B, C, N).astype(ml_dtypes.bfloat16)
    wT_g = np.tile(
        np.ascontiguousarray(w_qkv.T).astype(ml_dtypes.bfloat16), (NCORES, 1)
    )
    wpT_g = np.tile(
        np.ascontiguousarray(w_proj.T[_WPT_PERM]).astype(ml_dtypes.bfloat16),
        (NCORES, 1),
    )
    bias_g = np.tile(np.ascontiguousarray(b_proj, dtype=np.float32), NCORES)
    mask_g = np.tile(_band_mask(), (NCORES, 1))
    return [xT_g, wT_g, wpT_g, bias_g, mask_g]


def time_kernel(inputs, reps=8):
    """Return per-exec wall times (s) with device-resident inputs."""
    import jax
    from jax.sharding import NamedSharding, PartitionSpec

    fn, mesh = _get_runner()
    args = global_inputs(
        np.asarray(inputs["x"], dtype=np.float32),
        np.asarray(inputs["w_qkv"], dtype=np.float32),
        np.asarray(inputs["w_proj"], dtype=np.float32),
        np.asarray(inputs["b_proj"], dtype=np.float32),
    )
    sh = NamedSharding(mesh, PartitionSpec("core"))
    dargs = [jax.device_put(a, sh) for a in args]
    jax.block_until_ready(fn(*dargs))  # warm/compile
    import time as _time

    ts = []
    for _ in range(reps):
        t0 = _time.perf_counter()
        jax.block_until_ready(fn(*dargs))
        ts.append(_time.perf_counter() - t0)
    return ts


TIME_REPS = 32  # kernel executions emitted back-to-back inside the timing NEFF


def _bass_kernel_timed(nc, xT, wT, wpT, bias, maskband):
    """TIME_REPS full kernel executions in one NEFF (one launch), so the
    per-launch runtime overhead amortizes and the timed quantity approaches
    true per-execution device time."""
    yT_d = nc.dram_tensor("yT", [BLOC, C, N], F32, kind="ExternalOutput")
    for _ in range(TIME_REPS):
        _emit_body(nc, xT, wT, wpT, bias, maskband, yT_d)
    return yT_d


def _get_timed_runner():
    if "fn_t" in _NC_CACHE:
        return _NC_CACHE["fn_t"], _NC_CACHE["mesh_t"]
    import jax
    from jax.experimental.shard_map import shard_map
    from jax.sharding import Mesh, PartitionSpec

    from concourse.bass2jax import bass_jit

    kern = bass_jit(_bass_kernel_timed)
    devices = jax.devices()[:NCORES]
    mesh = Mesh(np.asarray(devices), ("core",))
    P = PartitionSpec
    fn = jax.jit(
        shard_map(
            kern,
            mesh=mesh,
            in_specs=(P("core"),) * 5,
            out_specs=P("core"),
            check_rep=False,
        )
    )
    _NC_CACHE["fn_t"] = fn
    _NC_CACHE["mesh_t"] = mesh
    return fn, mesh


def time_kernel_pipelined(inputs, n=128, trials=5):
    """Amortized per-exec time. Each jitted call runs the kernel TIME_REPS
    times back-to-back inside one NEFF (single launch); n calls are
    dispatched without intermediate blocking and synced once, so both the
    fixed ~70 ms axon round-trip latency and the ~1.3 ms per-launch runtime
    overhead amortize away. Returns per-EXECUTION times (call time divided
    by TIME_REPS); still an upper bound on true device time."""
    import jax
    from jax.sharding import NamedSharding, PartitionSpec
    import time as _time

    fn, mesh = _get_timed_runner()
    args = global_inputs(
        np.asarray(inputs["x"], dtype=np.float32),
        np.asarray(inputs["w_qkv"], dtype=np.float32),
        np.asarray(inputs["w_proj"], dtype=np.float32),
        np.asarray(inputs["b_proj"], dtype=np.float32),
    )
    sh = NamedSharding(mesh, PartitionSpec("core"))
    dargs = [jax.device_put(a, sh) for a in args]
    jax.block_until_ready(fn(*dargs))  # warm/compile
    out = []
    for _ in range(trials):
        t0 = _time.perf_counter()
        rs = [fn(*dargs) for _ in range(n)]
        jax.block_until_ready(rs)
        dt = _time.perf_counter() - t0
        del rs
        out.append(dt / (n * TIME_REPS))
    return out


def host_inputs(x, w_qkv, w_proj, b_proj):
    import ml_dtypes

    wT = np.ascontiguousarray(w_qkv.T).astype(ml_dtypes.bfloat16)
    wpT = np.ascontiguousarray(w_proj.T[_WPT_PERM]).astype(ml_dtypes.bfloat16)
    maskband = _band_mask()
    bias = np.ascontiguousarray(b_proj, dtype=np.float32)
    in_maps = []
    for i in range(NCORES):
        xT = _prep_xT(x[BLOC * i : BLOC * (i + 1)]).astype(ml_dtypes.bfloat16)
        in_maps.append(
            {
                "xT": xT,
                "wT": wT,
                "wpT": wpT,
                "bias": bias,
                "maskband": maskband,
            }
        )
    return in_maps


def kernel(x, w_qkv, w_proj, b_proj, H=None, W=None):
    x = np.asarray(x, dtype=np.float32)
    w_qkv = np.asarray(w_qkv, dtype=np.float32)
    w_proj = np.asarray(w_proj, dtype=np.float32)
    b_proj = np.asarray(b_proj, dtype=np.float32)
    fn, _ = _get_runner()
    args = global_inputs(x, w_qkv, w_proj, b_proj)
    yT = np.asarray(fn(*args))  # [16, 768, 1024] (w-major tokens)
    y = np.ascontiguousarray(yT.transpose(0, 2, 1)).reshape(B, N, C)
    return _unpermute_y(y).astype(np.float32)


def kernel_spmd(x, w_qkv, w_proj, b_proj, H=None, W=None):
    """Fallback path via run_bass_kernel_spmd (uncached compile per call)."""
    x = np.asarray(x, dtype=np.float32)
    w_qkv = np.asarray(w_qkv, dtype=np.float32)
    w_proj = np.asarray(w_proj, dtype=np.float32)
    b_proj = np.asarray(b_proj, dtype=np.float32)
    nc = _get_nc()
    in_maps = host_inputs(x, w_qkv, w_proj, b_proj)
    res = run_bass_kernel_spmd(nc, in_maps, list(range(NCORES)))
    yT = np.stack([res.results[i]["yT"] for i in range(NCORES)])  # [8, 2, 768, 1024]
    y = np.ascontiguousarray(yT.transpose(0, 1, 3, 2)).reshape(B, N, C)
    return _unpermute_y(y).astype(np.float32)


# revision 67
# speedup vs baseline: 1.0100x; 1.0100x over previous
"""Trainium2 Bass kernel for local-window multi-head self-attention (v6).

Problem shape (hardcoded): B=16, H=8, W=128 -> N=1024, C=768, nh=8, hd=96,
local window 7x11 (|dh|<=3, |dw|<=5).

v6 = v3 (w-major band attention, host-side transposes, hand interleave)
plus full-width PE tiling and a reworked pipeline:
  - Q/K projection runs with M=128 stationary tiles (12 row-groups over
    the 1536 q|k rows, single full-width eviction each) instead of
    per-head M=96 tiles: 73.7k instead of 98.3k PE columns per batch.
  - heads whose 96 q/k rows misalign with the 128-row packing are
    DMA-repacked (SBUF->SBUF, partition shift) into base-0 [96, N] tiles
    one head ahead of use, so every score matmul stays one instruction.
  - attention output is normalized into six packed [128, N] tiles with
    quadrant-legal DVE pieces; the output projection contracts over 6
    full 128-row chunks (73.7k -> 61.4k... 6 vs 8 matmuls per chunk).
  - head->outC slot permutation (wpT rows permuted on host to match)
    gives the late-emitted heads the 1-piece normalization slots, so the
    kernel tail never waits long DVE chains.
  - normalization multiplies are deferred one head (the reciprocal ->
    Pool broadcast chain never head-of-line blocks the DVE queue).
  - schedule: per-head ATT interleaves one qk unit (produced one position
    ahead of need) + spread PROJ(prev batch) chunks; all 16 V groups of
    the next batch fill positions 4-7; the final PROJ is software
    pipelined across 4 PSUM tiles and 4 output staging buffers.

Sharding: data-parallel over B across 8 NeuronCores (2 batches per core).
"""

import sys

sys.path.insert(0, "/opt/trn_rl_repo")

import numpy as np

import concourse.bacc as bacc
import concourse.mybir as mybir
import concourse.tile as tile
from concourse.bass_utils import run_bass_kernel_spmd

F32R = mybir.dt.float32r
F32 = mybir.dt.float32
BF16 = mybir.dt.bfloat16
AF = mybir.ActivationFunctionType

B, H, W, C = 16, 8, 128, 768
N = H * W  # 1024
NH, HD = 8, 96
NCORES = 8
BLOC = B // NCORES  # batches per core
SCALE = float(HD) ** -0.5
DH, DW = 3, 5  # |dh|<=3 rows, |dw|<=5 cols
QLO, QHI = 8 * DW, 128 + 8 * DW  # query window [128k-40, 128k+168)
MW = QLO + QHI  # mask width 208


def _att_blocks():
    """Emission-ordered key blocks: [(k, q0, q1, pieces)], order [0,1,7,2..6].

    k=0 (k=7) opens bank A (B) with a full-width 512-col AV matmul (its exm
    tile is zero-padded) so the start=True matmul covers the whole bank.
    Each piece: (c0, c1, half, start, stop) -- absolute query cols [c0, c1).
    """
    order = [0, 1, 7, 2, 3, 4, 5, 6]
    raw = {}
    for k in range(8):
        q0 = max(0, 128 * k - QLO)
        q1 = min(N, 128 * k + QHI)
        pieces = []
        if q0 < 512:
            pieces.append([q0, min(q1, 512), 0])
        if q1 > 512:
            pieces.append([max(q0, 512), q1, 1])
        raw[k] = (q0, q1, pieces)
    last_pos = {}
    for pos, k in enumerate(order):
        for _c0, _c1, half in raw[k][2]:
            last_pos[half] = pos
    blocks = []
    first = {0: True, 1: True}
    for pos, k in enumerate(order):
        q0, q1, pieces = raw[k]
        out = []
        for c0, c1, half in pieces:
            if first[half]:
                out.append((512 * half, 512 * half + 512, half, True, pos == last_pos[half]))
                first[half] = False
            else:
                out.append((c0, c1, half, False, pos == last_pos[half]))
        blocks.append((k, q0, q1, out))
    return blocks


ATT_BLOCKS = _att_blocks()


def build_nc():
    nc = bacc.Bacc(None, target_bir_lowering=False)
    xT_d = nc.dram_tensor("xT", [BLOC, C, N], BF16, kind="ExternalInput")
    wT_d = nc.dram_tensor("wT", [C, 3 * C], BF16, kind="ExternalInput")
    wpT_d = nc.dram_tensor("wpT", [C, C], BF16, kind="ExternalInput")
    bias_d = nc.dram_tensor("bias", [C], F32, kind="ExternalInput")
    mask_d = nc.dram_tensor("maskband", [128, MW], BF16, kind="ExternalInput")
    yT_d = nc.dram_tensor("yT", [BLOC, C, N], F32, kind="ExternalOutput")
    _emit_body(nc, xT_d, wT_d, wpT_d, bias_d, mask_d, yT_d)
    nc.finalize()
    return nc


def _emit_body(nc, xT_d, wT_d, wpT_d, bias_d, mask_d, yT_d):
    with tile.TileContext(nc) as tc:
        with (
            tc.tile_pool(name="const", bufs=1) as constp,
            tc.tile_pool(name="wperm", bufs=1) as wpermp,
            tc.tile_pool(name="xpool", bufs=2) as xp,
            tc.tile_pool(name="qkpool", bufs=1) as qkp,
            tc.tile_pool(name="vpool", bufs=2) as vp,
            tc.tile_pool(name="outp", bufs=2) as outp,
            tc.tile_pool(name="work", bufs=2) as workp,
            tc.tile_pool(name="ypool", bufs=4) as yp,
            tc.tile_pool(name="mmps", bufs=2, space="PSUM") as mmps,
            tc.tile_pool(name="scps", bufs=2, space="PSUM") as scps,
            tc.tile_pool(name="avps", bufs=2, space="PSUM") as avps,
        ):
            # ---- constants ----
            mask = constp.tile([128, MW], BF16, tag="mask", name="mask")
            nc.sync.dma_start(mask[:], mask_d[:])
            bias = constp.tile([128, 6], F32, tag="bias", name="bias")
            nc.sync.dma_start(bias[:], bias_d.ap().rearrange("(j p) -> p j", p=128))

            # zero-padded exm tiles for the bank-opening AV matmuls (k=0, k=7)
            exm_pad = [
                workp.tile([128, 512], BF16, tag=f"exmpad{i}", name=f"exmpad{i}", bufs=1)
                for i in range(2)
            ]
            nc.gpsimd.memset(exm_pad[0][:, QHI - QLO :], 0.0)
            nc.gpsimd.memset(exm_pad[1][:, : 512 - (QHI - QLO)], 0.0)

            # ---- weights: direct DMA of host-transposed layouts ----
            wT = [wpermp.tile([128, 3 * C], BF16, tag=f"wT{c}", name=f"wT{c}") for c in range(6)]
            wpT = [wpermp.tile([128, C], BF16, tag=f"wpT{j}", name=f"wpT{j}") for j in range(6)]

            def stage_x(b, with_wv=False):
                """Stage xT in token halves (and wv in ng halves) so the first
                DMA wave [x half0 + wv ng0] unblocks 8 full V groups at once
                instead of the c-loop crawling behind a serial DMA queue."""
                xT = [xp.tile([128, N], BF16, tag=f"xT{c}", name=f"xT{c}") for c in range(6)]
                for c in range(6):
                    nc.sync.dma_start(
                        xT[c][:, 0:512], xT_d[b, 128 * c : 128 * (c + 1), 0:512]
                    )
                    if with_wv:
                        nc.sync.dma_start(
                            wT[c][:, 2 * C : 2 * C + 384],
                            wT_d[128 * c : 128 * (c + 1), 2 * C : 2 * C + 384],
                        )
                for c in range(6):
                    nc.sync.dma_start(
                        xT[c][:, 512:1024], xT_d[b, 128 * c : 128 * (c + 1), 512:1024]
                    )
                if with_wv:
                    for c in range(6):
                        nc.sync.dma_start(
                            wT[c][:, 2 * C + 384 :],
                            wT_d[128 * c : 128 * (c + 1), 2 * C + 384 :],
                        )
                return xT

            def load_weights():
                for c in range(6):
                    nc.sync.dma_start(
                        wT[c][:, : 2 * C], wT_d[128 * c : 128 * (c + 1), : 2 * C]
                    )
                for j in range(6):
                    nc.sync.dma_start(wpT[j][:], wpT_d[128 * j : 128 * (j + 1), :])

            def v_groups(xT):
                """16 closures, each a 6-MM group computing one v_sb chunk."""
                v_sb = vp.tile([128, 8 * NH * 97], BF16, tag="v", name="v")
                ones_ap = v_sb[:].rearrange("p (t e) -> p t e", t=64)[:, :, 96:97]
                nc.gpsimd.memset(ones_ap, 1.0)
                groups = []
                for ng in range(2):
                    for t in range(8):
                        def g(t=t, ng=ng):
                            pv = mmps.tile([128, 384], F32, tag="mm", name="mm")
                            for c in range(6):
                                nc.tensor.matmul(
                                    pv[:],
                                    xT[c][:, 128 * t : 128 * (t + 1)],
                                    wT[c][:, 2 * C + 384 * ng : 2 * C + 384 * (ng + 1)],
                                    start=(c == 0),
                                    stop=(c == 5),
                                )
                            out_ap = v_sb[:].rearrange("p (t h e) -> p t h e", t=8, h=NH)[
                                :, t, 4 * ng : 4 * (ng + 1), 0:96
                            ]
                            nc.scalar.copy(
                                out_ap, pv[:].rearrange("p (h e) -> p h e", h=4)
                            )
                        groups.append(g)
                return v_sb, groups

            def qk_tiles():
                """12 packed [128, N] bf16 tiles holding the q|k rows
                [128g, 128g+128); aligned heads (base 0) are read directly,
                misaligned heads through the DMA-repacked qa/ka tiles."""
                return [
                    qkp.tile([128, N], BF16, tag=f"qk{g}", name=f"qk{g}")
                    for g in range(12)
                ]

            STRADDLE = (1, 2, 3, 5, 6, 7)  # heads whose rows straddle/misalign

            # head -> outC slot, by normalization piece count (slot base
            # 0/32/64/96 -> 1/3/2/3 quadrant pieces): late-emitted heads get
            # the cheap slots (h7,h4 -> 1 piece; h5,h6 -> 2) so the tail DVE
            # queue stays short; wpT rows are permuted to match on the host.
            SLOT = {7: 0, 0: 1, 5: 2, 1: 3, 4: 4, 2: 5, 6: 6, 3: 7}

            def qa_tiles():
                qa = {h: qkp.tile([HD, N], BF16, tag=f"qa{h}", name=f"qa{h}") for h in STRADDLE}
                ka = {h: qkp.tile([HD, N], BF16, tag=f"ka{h}", name=f"ka{h}") for h in STRADDLE}
                return qa, ka

            def emit_repack(h, qk_sb, qa, ka):
                """DMA the misaligned q/k rows of head h out of the packed
                tiles into partition-base-0 [96, N] tiles (Act HWDGE queue,
                issued one full head ahead of use)."""
                for dst, base_e in ((qa[h], 96 * h), (ka[h], 768 + 96 * h)):
                    r = base_e
                    while r < base_e + 96:
                        g = r // 128
                        r1 = min(base_e + 96, 128 * (g + 1))
                        nc.sync.dma_start(
                            dst[r - base_e : r1 - base_e, :],
                            qk_sb[g][r - 128 * g : r1 - 128 * g, :],
                        )
                        r = r1

            def qk_group_closures(xT, qk_sb, g):
                """Two closures (one per token half): 6-MM M=128 row-group g of
                the packed q|k rows, evicted with one full-width copy."""
                out = []
                for half in (0, 1):
                    def go(half=half, g=g):
                        pq = mmps.tile([128, 512], F32, tag="mm", name="mm")
                        for c in range(6):
                            nc.tensor.matmul(
                                pq[:],
                                wT[c][:, 128 * g : 128 * (g + 1)],
                                xT[c][:, 512 * half : 512 * (half + 1)],
                                start=(c == 0),
                                stop=(c == 5),
                            )
                        nc.scalar.copy(
                            qk_sb[g][:, 512 * half : 512 * (half + 1)], pq[:]
                        )
                    out.append(go)
                return out

            def proj_closures(b, outC):
                """12 closures: output projection chunk (e, half) with K=128
                contraction over the 6 packed outC tiles. half-major order so
                the half-0 chunks never wait on NORM(*, 1); bias-add eviction
                alternates Act/DVE to halve eviction back-pressure."""
                out = []
                for half in range(2):
                    for e in range(6):
                        def go(e=e, half=half, outC=outC, b=b):
                            py = mmps.tile([128, 512], F32, tag="mm", name="mm")
                            # j=0/3 last: finished by the tail heads (h7/h4)
                            for ji, j in enumerate((1, 2, 4, 5, 0, 3)):
                                nc.tensor.matmul(
                                    py[:],
                                    wpT[j][:, 128 * e : 128 * (e + 1)],
                                    outC[j][:, 512 * half : 512 * (half + 1)],
                                    start=(ji == 0),
                                    stop=(ji == 5),
                                )
                            yt = yp.tile([128, 512], F32, tag="yt", name="yt")
                            nc.scalar.add(yt[:], py[:], bias[:, e : e + 1])
                            nc.sync.dma_start(
                                yT_d[b, 128 * e : 128 * (e + 1), 512 * half : 512 * (half + 1)],
                                yt[:],
                            )
                        out.append(go)
                return out

            def emit_att(h, qsrc, ksrc, v_sb, outC, fillers, pending, last=False):
                """ATT(b,h) with filler closures spliced into the PE stream.

                The outC normalization multiplies of head h-1 arrive as
                `pending` and are emitted late in this head's sequence, after
                this head's mask-muls: the reciprocal -> Pool broadcast chain
                then never head-of-line blocks the DVE queue. Returns this
                head's deferred multiplies."""
                av = [avps.tile([97, 512], F32, tag=f"av{i}", name=f"av{i}") for i in range(2)]
                sc_t = {}
                exm_t = {}
                qt, qr = qsrc
                kt, kr = ksrc

                def S(j):
                    k, q0, q1, _p = ATT_BLOCKS[j]
                    wq = q1 - q0
                    mo = q0 - (128 * k - QLO)
                    sc = scps.tile([128, 256], F32, tag="sc", name="sc")
                    sc_t[j] = sc
                    nc.tensor.matmul(
                        sc[:, :wq],
                        kt[kr : kr + HD, 128 * k : 128 * (k + 1)],
                        qt[qr : qr + HD, q0:q1],
                        start=True,
                        stop=True,
                    )
                    ex = workp.tile([128, 256], BF16, tag="ex", name="ex", bufs=4)
                    nc.scalar.activation(ex[:, :wq], sc[:, :wq], AF.Exp, scale=SCALE)
                    if k in (0, 7):
                        exm = exm_pad[0 if k == 0 else 1]
                        eo = q0 - 512 * (k == 7)
                    else:
                        exm = workp.tile([128, 256], BF16, tag="exm", name="exm", bufs=4)
                        eo = 0
                    nc.vector.tensor_mul(
                        exm[:, eo : eo + wq], ex[:, :wq], mask[:, mo : mo + wq]
                    )
                    exm_t[j] = (exm, eo)

                def A(j):
                    k, q0, q1, pieces = ATT_BLOCKS[j]
                    exm, eo = exm_t[j]
                    vs = v_sb[:].rearrange("p (t e) -> p t e", t=64)[:, k * NH + h, :]
                    for c0, c1, half, start, stop in pieces:
                        if k in (0, 7):
                            rhs = exm[:, c0 - 512 * half : c1 - 512 * half]
                        else:
                            rhs = exm[:, c0 - q0 + eo : c1 - q0 + eo]
                        nc.tensor.matmul(
                            av[half][:, c0 - 512 * half : c1 - 512 * half],
                            vs,
                            rhs,
                            start=start,
                            stop=stop,
                        )

                deferred = []

                def NORM_pre(half):
                    rec = workp.tile([1, 512], F32, tag="rec", name="rec")
                    nc.vector.reciprocal(rec[:], av[half][96:97, :])
                    recb = workp.tile([HD, 512], F32, tag="recb", name="recb", bufs=3)
                    nc.gpsimd.partition_broadcast(recb[:], rec[:])

                    CAP = {0: 128, 32: 32, 64: 64, 96: 32}
                    s = SLOT[h]

                    def muls(half=half, recb=recb, av=av, outC=outC, s=s):
                        r = 96 * s
                        while r < 96 * (s + 1):
                            j = r // 128
                            ob, ib = r - 128 * j, r - 96 * s
                            step = min(
                                96 * (s + 1) - r, 128 * (j + 1) - r, CAP[ob], CAP[ib]
                            )
                            nc.vector.tensor_mul(
                                outC[j][ob : ob + step, 512 * half : 512 * (half + 1)],
                                av[half][ib : ib + step, :],
                                recb[ib : ib + step, :],
                            )
                            r += step

                    deferred.append(muls)

                fi = iter(fillers)

                def F():
                    g = next(fi, None)
                    if g is not None:
                        g()

                def PENDING():
                    for m in pending:
                        m()

                def PEND1(i):
                    if i < len(pending):
                        pending[i]()

                if not last:
                    seq = [
                        lambda: S(0), lambda: S(1), F, lambda: A(0),
                        lambda: S(2), F, lambda: A(1),
                        lambda: S(3), F, lambda: A(2),
                        lambda: S(4), lambda: A(3),
                        lambda: S(5), F, lambda: A(4),
                        lambda: S(6), lambda: S(7), F, lambda: A(5),
                        PENDING, lambda: NORM_pre(0), lambda: A(6), lambda: A(7),
                        lambda: NORM_pre(1),
                    ]
                else:
                    # final head: exp-paced compact ordering, fillers pushed
                    # to the drain, so bank A finishes ~2us earlier and the
                    # normalization chain (reciprocal -> Pool broadcast ->
                    # mul) hides under the first PROJ chunks
                    seq = [
                        lambda: S(0), lambda: S(1), F, lambda: A(0),
                        lambda: S(2), lambda: A(1),
                        lambda: S(3), lambda: A(2),
                        lambda: S(4), lambda: A(3),
                        lambda: S(5), lambda: A(4),
                        lambda: S(6), lambda: S(7), lambda: A(5),
                        lambda: NORM_pre(0), lambda: PEND1(0), lambda: deferred[0](),
                        lambda: A(6), lambda: A(7),
                        lambda: NORM_pre(1), lambda: PEND1(1), lambda: deferred[1](),
                    ]
                for step in seq:
                    step()
                # drain any unused fillers
                for g in fi:
                    g()
                return [] if last else deferred

            # ================= main schedule =================
            xT = stage_x(0, with_wv=True)
            load_weights()
            v_sb, vgs = v_groups(xT)
            for g in vgs:
                g()
            next_xT = None
            next_v = None
            prev_proj = None
            pending = []
            for b in range(BLOC):
                if b > 0:
                    xT, v_sb = next_xT, next_v
                qk_sb = qk_tiles()
                qa, ka = qa_tiles()
                outC = [
                    outp.tile([128, N], BF16, tag=f"outC{j}", name=f"outC{j}")
                    for j in range(6)
                ]
                for g0 in (0, 6, 1, 7):
                    for go in qk_group_closures(xT, qk_sb, g0):
                        go()
                if b + 1 < BLOC:
                    next_xT = stage_x(b + 1)
                # Head emission order: h4 last (its 96 rows sit at partition
                # base 0 of outC3, so the tail normalization is one piece).
                # Position p's unit produces the qk tiles position p+2 needs;
                # position p also issues the repack DMAs for position p+1.
                HEAD_ORDER = (0, 1, 2, 3, 5, 6, 7, 4)
                UNITS = {0: (2, 8), 1: (3, 9), 2: (4, 10), 3: (5, 11)}
                # PROJ(b-1) chunks spread over positions 1-7 (position 0 emits
                # the deferred h7 normalization PENDING, which PROJ(b-1) reads)
                pchunks = {0: [], 1: [0, 1], 2: [2], 3: [3], 4: [4, 5],
                           5: [6, 7], 6: [8, 9], 7: [10, 11]}

                def src(h):
                    if h in qa:
                        return (qa[h], 0), (ka[h], 0)
                    g = (96 * h) // 128
                    return (qk_sb[g], (96 * h) % 128), (qk_sb[g + 6], (96 * h) % 128)

                for pos, h in enumerate(HEAD_ORDER):
                    if pos + 1 < NH and HEAD_ORDER[pos + 1] in qa:
                        emit_repack(HEAD_ORDER[pos + 1], qk_sb, qa, ka)
                    if pos < 4:
                        g1, g2 = UNITS[pos]
                        fillers = qk_group_closures(xT, qk_sb, g1) + qk_group_closures(
                            xT, qk_sb, g2
                        )
                        if prev_proj is not None:
                            fillers += [prev_proj[i] for i in pchunks[pos]]
                    elif b + 1 < BLOC:
                        if pos == 4:
                            next_v, nvgs = v_groups(next_xT)
                            fillers = nvgs[:4]
                        elif pos == 5:
                            fillers = nvgs[4:9]
                        elif pos == 6:
                            fillers = nvgs[9:13]
                        else:
                            fillers = nvgs[13:]
                    elif prev_proj is not None:
                        fillers = [prev_proj[i] for i in pchunks[pos]]
                    else:
                        fillers = []
                    last = b + 1 == BLOC and pos == 7
                    qsrc, ksrc = src(h)
                    pending = emit_att(
                        h, qsrc, ksrc, v_sb, outC, fillers, pending, last=last
                    )
                if b + 1 < BLOC:
                    prev_proj = proj_closures(b, outC)
                else:
                    # software-pipelined final PROJ: each chunk's j=3 matmul
                    # (gated on the last head's normalization) trails one
                    # chunk of independent work
                    chunks = [(half, e) for half in range(2) for e in range(6)]
                    ptile = {}

                    def PA(i):
                        half, e = chunks[i]
                        # alternate between the mm pool and the (now idle) av
                        # pool: 4 PSUM tiles deep, eviction latency fully hidden
                        if i % 2 == 0:
                            py = mmps.tile([128, 512], F32, tag="mm", name="mm")
                        else:
                            py = avps.tile([128, 512], F32, tag="av0", name="av0")
                        ptile[i] = py
                        for ji, j in enumerate((1, 2, 4, 5)):
                            nc.tensor.matmul(
                                py[:],
                                wpT[j][:, 128 * e : 128 * (e + 1)],
                                outC[j][:, 512 * half : 512 * (half + 1)],
                                start=(ji == 0),
                                stop=False,
                            )

                    def PB(i):
                        half, e = chunks[i]
                        py = ptile.pop(i)
                        for ji, j in enumerate((0, 3)):
                            nc.tensor.matmul(
                                py[:],
                                wpT[j][:, 128 * e : 128 * (e + 1)],
                                outC[j][:, 512 * half : 512 * (half + 1)],
                                start=False,
                                stop=(ji == 1),
                            )
                        yt = yp.tile([128, 512], F32, tag="yt", name="yt")
                        if i % 2 == 0:
                            nc.scalar.add(yt[:], py[:], bias[:, e : e + 1])
                        else:
                            nc.vector.tensor_scalar_add(yt[:], py[:], bias[:, e : e + 1])
                        nc.sync.dma_start(
                            yT_d[b, 128 * e : 128 * (e + 1), 512 * half : 512 * (half + 1)],
                            yt[:],
                        )

                    PA(0)
                    for i in range(1, 12):
                        PA(i)
                        PB(i - 1)
                    PB(11)


_NC_CACHE = {}


def _get_nc():
    if "nc" not in _NC_CACHE:
        _NC_CACHE["nc"] = build_nc()
    return _NC_CACHE["nc"]


def _bass_kernel(nc, xT, wT, wpT, bias, maskband):
    yT_d = nc.dram_tensor("yT", [BLOC, C, N], F32, kind="ExternalOutput")
    _emit_body(nc, xT, wT, wpT, bias, maskband, yT_d)
    return yT_d


def _get_runner():
    if "fn" in _NC_CACHE:
        return _NC_CACHE["fn"], _NC_CACHE["mesh"]
    import jax
    from jax.experimental.shard_map import shard_map
    from jax.sharding import Mesh, PartitionSpec

    from concourse.bass2jax import bass_jit

    kern = bass_jit(_bass_kernel)
    devices = jax.devices()[:NCORES]
    mesh = Mesh(np.asarray(devices), ("core",))
    P = PartitionSpec
    fn = jax.jit(
        shard_map(
            kern,
            mesh=mesh,
            in_specs=(P("core"),) * 5,
            out_specs=P("core"),
            check_rep=False,
        )
    )
    _NC_CACHE["fn"] = fn
    _NC_CACHE["mesh"] = mesh
    return fn, mesh


# outC slot s holds head _SLOT_HEADS[s]'s output dims (see SLOT in the
# kernel body); wpT rows are permuted to match.
_SLOT_HEADS = (7, 0, 5, 1, 4, 2, 6, 3)
_WPT_PERM = np.concatenate([np.arange(96 * h, 96 * (h + 1)) for h in _SLOT_HEADS])


def _band_mask():
    """[128, 208] bf16: mask[i, j] for key i in block, query offset r=j-40."""
    import ml_dtypes

    i = np.arange(128)
    r = np.arange(-QLO, QHI)
    wk, hk = i // 8, i % 8
    wq, hq = np.floor_divide(r, 8), np.mod(r, 8)
    m = (np.abs(wk[:, None] - wq[None, :]) <= DW) & (
        np.abs(hk[:, None] - hq[None, :]) <= DH
    )
    return m.astype(np.float32).astype(ml_dtypes.bfloat16)


def _prep_xT(x):
    """[Bn, N, C] row-major tokens -> [Bn, C, N'] with w-major tokens."""
    Bn = x.shape[0]
    return np.ascontiguousarray(
        x.reshape(Bn, H, W, C).transpose(0, 3, 2, 1).reshape(Bn, C, N)
    )


def _unpermute_y(y):
    """w-major tokens back to row-major."""
    Bn = y.shape[0]
    return np.ascontiguousarray(
        y.reshape(Bn, W, H, C).transpose(0, 2, 1, 3).reshape(Bn, N, C)
    )


def global_inputs(x, w_qkv, w_proj, b_proj):
    """Pre-process + concatenate per-core inputs along axis 0 for shard_map."""
    import ml_dtypes

    xT_g = _prep_xT(x).reshape(B, C, N).astype(ml_dtypes.bfloat16)
    wT_g = np.tile(
        np.ascontiguousarray(w_qkv.T).astype(ml_dtypes.bfloat16), (NCORES, 1)
    )
    wpT_g = np.tile(
        np.ascontiguousarray(w_proj.T[_WPT_PERM]).astype(ml_dtypes.bfloat16),
        (NCORES, 1),
    )
    bias_g = np.tile(np.ascontiguousarray(b_proj, dtype=np.float32), NCORES)
    mask_g = np.tile(_band_mask(), (NCORES, 1))
    return [xT_g, wT_g, wpT_g, bias_g, mask_g]


def time_kernel(inputs, reps=8):
    """Return per-exec wall times (s) with device-resident inputs."""
    import jax
    from jax.sharding import NamedSharding, PartitionSpec

    fn, mesh = _get_runner()
    args = global_inputs(
        np.asarray(inputs["x"], dtype=np.float32),
        np.asarray(inputs["w_qkv"], dtype=np.float32),
        np.asarray(inputs["w_proj"], dtype=np.float32),
        np.asarray(inputs["b_proj"], dtype=np.float32),
    )
    sh = NamedSharding(mesh, PartitionSpec("core"))
    dargs = [jax.device_put(a, sh) for a in args]
    jax.block_until_ready(fn(*dargs))  # warm/compile
    import time as _time

    ts = []
    for _ in range(reps):
        t0 = _time.perf_counter()
        jax.block_until_ready(fn(*dargs))
        ts.append(_time.perf_counter() - t0)
    return ts


TIME_REPS = 32  # kernel executions emitted back-to-back inside the timing NEFF


def _bass_kernel_timed(nc, xT, wT, wpT, bias, maskband):
    """TIME_REPS full kernel executions in one NEFF (one launch), so the
    per-launch runtime overhead amortizes and the timed quantity approaches
    true per-execution device time."""
    yT_d = nc.dram_tensor("yT", [BLOC, C, N], F32, kind="ExternalOutput")
    for _ in range(TIME_REPS):
        _emit_body(nc, xT, wT, wpT, bias, maskband, yT_d)
    return yT_d


def _get_timed_runner():
    if "fn_t" in _NC_CACHE:
        return _NC_CACHE["fn_t"], _NC_CACHE["mesh_t"]
    import jax
    from jax.experimental.shard_map import shard_map
    from jax.sharding import Mesh, PartitionSpec

    from concourse.bass2jax import bass_jit

    kern = bass_jit(_bass_kernel_timed)
    devices = jax.devices()[:NCORES]
    mesh = Mesh(np.asarray(devices), ("core",))
    P = PartitionSpec
    fn = jax.jit(
        shard_map(
            kern,
            mesh=mesh,
            in_specs=(P("core"),) * 5,
            out_specs=P("core"),
            check_rep=False,
        )
    )
    _NC_CACHE["fn_t"] = fn
    _NC_CACHE["mesh_t"] = mesh
    return fn, mesh


def time_kernel_pipelined(inputs, n=128, trials=5):
    """Amortized per-exec time. Each jitted call runs the kernel TIME_REPS
    times back-to-back inside one NEFF (single launch); n calls are
    dispatched without intermediate blocking and synced once, so both the
    fixed ~70 ms axon round-trip latency and the ~1.3 ms per-launch runtime
    overhead amortize away. Returns per-EXECUTION times (call time divided
    by TIME_REPS); still an upper bound on true device time."""
    import jax
    from jax.sharding import NamedSharding, PartitionSpec
    import time as _time

    fn, mesh = _get_timed_runner()
    args = global_inputs(
        np.asarray(inputs["x"], dtype=np.float32),
        np.asarray(inputs["w_qkv"], dtype=np.float32),
        np.asarray(inputs["w_proj"], dtype=np.float32),
        np.asarray(inputs["b_proj"], dtype=np.float32),
    )
    sh = NamedSharding(mesh, PartitionSpec("core"))
    dargs = [jax.device_put(a, sh) for a in args]
    jax.block_until_ready(fn(*dargs))  # warm/compile
    out = []
    for _ in range(trials):
        t0 = _time.perf_counter()
        rs = [fn(*dargs) for _ in range(n)]
        jax.block_until_ready(rs)
        dt = _time.perf_counter() - t0
        del rs
        out.append(dt / (n * TIME_REPS))
    return out


def host_inputs(x, w_qkv, w_proj, b_proj):
    import ml_dtypes

    wT = np.ascontiguousarray(w_qkv.T).astype(ml_dtypes.bfloat16)
    wpT = np.ascontiguousarray(w_proj.T[_WPT_PERM]).astype(ml_dtypes.bfloat16)
    maskband = _band_mask()
    bias = np.ascontiguousarray(b_proj, dtype=np.float32)
    in_maps = []
    for i in range(NCORES):
        xT = _prep_xT(x[BLOC * i : BLOC * (i + 1)]).astype(ml_dtypes.bfloat16)
        in_maps.append(
            {
                "xT": xT,
                "wT": wT,
                "wpT": wpT,
                "bias": bias,
                "maskband": maskband,
            }
        )
    return in_maps


def kernel(x, w_qkv, w_proj, b_proj, H=None, W=None):
    x = np.asarray(x, dtype=np.float32)
    w_qkv = np.asarray(w_qkv, dtype=np.float32)
    w_proj = np.asarray(w_proj, dtype=np.float32)
    b_proj = np.asarray(b_proj, dtype=np.float32)
    fn, _ = _get_runner()
    args = global_inputs(x, w_qkv, w_proj, b_proj)
    yT = np.asarray(fn(*args))  # [16, 768, 1024] (w-major tokens)
    y = np.ascontiguousarray(yT.transpose(0, 2, 1)).reshape(B, N, C)
    return _unpermute_y(y).astype(np.float32)


def kernel_spmd(x, w_qkv, w_proj, b_proj, H=None, W=None):
    """Fallback path via run_bass_kernel_spmd (uncached compile per call)."""
    x = np.asarray(x, dtype=np.float32)
    w_qkv = np.asarray(w_qkv, dtype=np.float32)
    w_proj = np.asarray(w_proj, dtype=np.float32)
    b_proj = np.asarray(b_proj, dtype=np.float32)
    nc = _get_nc()
    in_maps = host_inputs(x, w_qkv, w_proj, b_proj)
    res = run_bass_kernel_spmd(nc, in_maps, list(range(NCORES)))
    yT = np.stack([res.results[i]["yT"] for i in range(NCORES)])  # [8, 2, 768, 1024]
    y = np.ascontiguousarray(yT.transpose(0, 1, 3, 2)).reshape(B, N, C)
    return _unpermute_y(y).astype(np.float32)
